# revision 11
# baseline (speedup 1.0000x reference)
"""Trainium2 Bass kernel for a BiLSTM dependency-parse model.

Computes, on 8 NeuronCores (SPMD, per-core input data differences only):
  x  = concat(emb_word[words], emb_tag[tags])           [256, 400]
  hv = 2-layer BiLSTM(x)                                [256, 800]
  scores[i,j] = aW2 . tanh(Ah[i] + Ad[j] + ab1) (+mask) [256, 256]
  labels      = (tanh(hv@lW1h.T + hv[arcs]@lW1d.T + lb1)) @ lW2.T + lb2

Sharding: the LSTM runs replicated on every core; the [256,256] arc grid is
row-sharded (32 head rows per core) via per-core one-hot selection inputs;
labels are computed replicated (cheap) and taken from core 0.

Layouts (device):
  hidden vectors v[400]   -> [100 part, 4 cols]   (v[100*c+p])
  gate vectors  g[1600]   -> [100 part, 16 cols]  gate order [i,f,o,g]
  sequences     hvT        -> [100, 8, 256]       (BI=800 on partitions)
  MLPIN k=1600            -> [128 part, 13 chunks] (tail chunk 64 valid)
"""
import sys
sys.path.insert(0, "/opt/trn_rl_repo")

import numpy as np
import ml_dtypes

from concourse import bass, bacc, mybir
from concourse.bass import ds, IndirectOffsetOnAxis
from concourse.tile import TileContext
from concourse.bass_utils import run_bass_kernel_spmd

T = 256
V = 50000
NTAGS = 50
L = 40
WD = 300
TD = 100
IN = 400
H = 400
BI = 800
MLPIN = 1600
PB = 100          # partition size for hidden/gate layouts
NC_ = 8           # cores
RB = T // NC_     # 32 score rows per core
KC13 = 13         # MLPIN chunks of 128 (last 64 valid)
F32 = mybir.dt.float32
BF16 = mybir.dt.bfloat16
I32 = mybir.dt.int32
AF = mybir.ActivationFunctionType
OP = mybir.AluOpType
BF = ml_dtypes.bfloat16

# new gate order [i, f, o, g]; offsets into the pytorch [i,f,g,o] row layout
OLD_OFF = [0, 400, 1200, 800]


def _perm_rows(M):
    """[1600, ...] pytorch-gate-ordered -> [16, 100, ...] new-layout blocks."""
    out = np.empty((16, 100) + M.shape[1:], M.dtype)
    for mc in range(16):
        gate, c = mc // 4, mc % 4
        r0 = OLD_OFF[gate] + c * 100
        out[mc] = M[r0:r0 + 100]
    return out


def _w_stat(W):
    """W [1600, KIN*100] -> lhsT stat blocks [100(k), KIN, 16, 100(m)] (part-major)."""
    kin = W.shape[1] // PB
    Wp = _perm_rows(W)                                   # [16, 100m, kin*100]
    S = Wp.reshape(16, 100, kin, 100)                    # [mc, m, kc, k]
    return np.ascontiguousarray(S.transpose(3, 2, 0, 1))  # [k, kc, mc, m]


def _bi_stat(W):
    """W [1600, 800] -> lhsT blocks [100(k), 8, 13, 128(m)], m-padded with 0."""
    Wp = np.zeros((KC13 * 128, BI), np.float32)
    Wp[:MLPIN] = W
    S = Wp.reshape(KC13, 128, 8, 100)                    # [mc, m, kc, k]
    return np.ascontiguousarray(S.transpose(3, 2, 0, 1))  # [k, kc, mc, m]


def _k13(v):
    """v [1600] -> [128, 13] (padded)."""
    p = np.zeros(KC13 * 128, np.float32)
    p[:MLPIN] = v
    return np.ascontiguousarray(p.reshape(KC13, 128).T)


def _hperm(v):
    """v [400] -> [100, 4]."""
    return np.ascontiguousarray(v.reshape(4, 100).T)


def _prep(inputs):
    """Host-side prep: returns (shared_map, per_core_maps)."""
    g = {k: np.asarray(v) for k, v in inputs.items()}
    f32 = lambda a: np.ascontiguousarray(a, np.float32)
    bf = lambda a: np.ascontiguousarray(a).astype(np.float32).astype(BF)

    sh = {}
    sh["emb_word"] = f32(g["emb_word"])
    sh["emb_tag"] = f32(g["emb_tag"])
    sh["widx"] = np.ascontiguousarray(g["words_tensor"].astype(np.int32).reshape(T, 1))
    sh["tidx"] = np.ascontiguousarray(g["tags_tensor"].astype(np.int32).reshape(T, 1))

    # LSTM weights: Wi_l [100, 2, KIN, 16, 100], Wr_l [100, 2, 4, 16, 100]
    for l, (Wih, Whh, bih, bhh) in enumerate(
        [(g["Wih0"], g["Whh0"], g["bih0"], g["bhh0"]),
         (g["Wih1"], g["Whh1"], g["bih1"], g["bhh1"])]):
        wi = np.stack([_w_stat(np.asarray(Wih[d], np.float32)) for d in range(2)], 1)
        wr = np.stack([_w_stat(np.asarray(Whh[d], np.float32)) for d in range(2)], 1)
        bs = np.stack(
            [_perm_rows(np.asarray(bih[d] + bhh[d], np.float32)).T for d in range(2)], 1)
        sh[f"Wi{l}"] = bf(wi)
        sh[f"Wr{l}"] = bf(wr)
        sh[f"bias{l}"] = f32(bs)                          # [100, 2, 16]
    h0, c0 = np.asarray(g["h0"], np.float32), np.asarray(g["c0"], np.float32)
    sh["h00"] = f32(np.stack([_hperm(h0[0]), _hperm(h0[1])], 1))   # [100, 2, 4]
    sh["c00"] = f32(np.stack([_hperm(c0[0]), _hperm(c0[1])], 1))
    sh["h01"] = f32(np.stack([_hperm(h0[2]), _hperm(h0[3])], 1))
    sh["c01"] = f32(np.stack([_hperm(c0[2]), _hperm(c0[3])], 1))

    aW1 = np.asarray(g["aW1"], np.float32)
    lW1 = np.asarray(g["lW1"], np.float32)
    sh["aW1h_s"] = bf(_bi_stat(aW1[:, :BI]))
    sh["aW1d_s"] = bf(_bi_stat(aW1[:, BI:]))
    sh["lW1h_s"] = bf(_bi_stat(lW1[:, :BI]))
    sh["lW1d_s"] = bf(_bi_stat(lW1[:, BI:]))
    sh["ab1T"] = f32(_k13(np.asarray(g["ab1"], np.float32)))
    sh["lb1T"] = f32(_k13(np.asarray(g["lb1"], np.float32)))
    sh["aW2T"] = bf(_k13(np.asarray(g["aW2"], np.float32)[0]))
    lW2p = np.zeros((L, KC13 * 128), np.float32)
    lW2p[:, :MLPIN] = np.asarray(g["lW2"], np.float32)
    sh["lW2T"] = bf(lW2p.reshape(L, KC13, 128).transpose(2, 1, 0))  # [128, 13, 40]
    sh["lb2b"] = f32(np.tile(np.asarray(g["lb2"], np.float32), (128, 1)))
    sh["ab2b"] = np.asarray(g["ab2"], np.float32).reshape(1, 1)

    arcs = np.asarray(g["arcs_refdata_tensor"]).astype(np.int64)
    A = np.zeros((T, T), np.float32)
    A[arcs, np.arange(T)] = 1.0                           # A[t', t] = 1 iff arcs[t]==t'
    sh["arcsel"] = bf(A.reshape(2, 128, 2, 128).transpose(1, 0, 2, 3))  # [128,tcp,tc,128]

    sh["ident_f"] = np.eye(128, dtype=np.float32)
    sh["ident_b"] = np.eye(128, dtype=np.float32).astype(BF)

    ab2 = float(np.asarray(g["ab2"], np.float32).reshape(-1)[0])

    per_core = []
    for c in range(NC_):
        m = {}
        I = np.zeros((T, RB), np.float32)
        I[np.arange(c * RB, (c + 1) * RB), np.arange(RB)] = 1.0
        m["isel"] = bf(I.reshape(2, 128, RB).transpose(1, 0, 2))   # [128, 2, 32]
        mA = np.ones((RB, T), np.float32)
        mA[np.arange(RB), np.arange(c * RB, (c + 1) * RB)] = 0.0
        mB = np.zeros((RB, T), np.float32)
        if c == 0:
            mB[0, 0] = 1.0
        m["maskA"], m["maskB"] = mA, mB
        per_core.append(m)
    return sh, per_core, ab2


def _build(ab2: float, unroll: int = 1):
    nc = bacc.Bacc(None, target_bir_lowering=False)
    P = lambda n, s, dt: nc.declare_dram_parameter(n, list(s), dt, isOutput=False)

    d_embw = P("emb_word", (V, WD), F32)
    d_embt = P("emb_tag", (NTAGS, TD), F32)
    d_widx = P("widx", (T, 1), I32)
    d_tidx = P("tidx", (T, 1), I32)
    d_Wi = [P("Wi0", (PB, 2, 4, 16, PB), BF16), P("Wi1", (PB, 2, 8, 16, PB), BF16)]
    d_Wr = [P("Wr0", (PB, 2, 4, 16, PB), BF16), P("Wr1", (PB, 2, 4, 16, PB), BF16)]
    d_bias = [P("bias0", (PB, 2, 16), F32), P("bias1", (PB, 2, 16), F32)]
    d_h0 = [P("h00", (PB, 2, 4), F32), P("h01", (PB, 2, 4), F32)]
    d_c0 = [P("c00", (PB, 2, 4), F32), P("c01", (PB, 2, 4), F32)]
    d_aW1h = P("aW1h_s", (PB, 8, KC13, 128), BF16)
    d_aW1d = P("aW1d_s", (PB, 8, KC13, 128), BF16)
    d_lW1h = P("lW1h_s", (PB, 8, KC13, 128), BF16)
    d_lW1d = P("lW1d_s", (PB, 8, KC13, 128), BF16)
    d_ab1T = P("ab1T", (128, KC13), F32)
    d_lb1T = P("lb1T", (128, KC13), F32)
    d_aW2T = P("aW2T", (128, KC13), BF16)
    d_lW2T = P("lW2T", (128, KC13, L), BF16)
    d_lb2b = P("lb2b", (128, L), F32)
    d_ab2b = P("ab2b", (1, 1), F32)
    d_arcsel = P("arcsel", (128, 2, 2, 128), BF16)
    d_identf = P("ident_f", (128, 128), F32)
    d_identb = P("ident_b", (128, 128), BF16)
    d_isel = P("isel", (128, 2, RB), BF16)
    d_maskA = P("maskA", (RB, T), F32)
    d_maskB = P("maskB", (RB, T), F32)

    o_scores = nc.declare_dram_parameter("scores_blk", [RB, T], F32, isOutput=True)
    o_labels = nc.declare_dram_parameter("labels_o", [T, L], F32, isOutput=True)

    with TileContext(nc) as tc:
        with (
            tc.tile_pool(name="wres", bufs=1) as wres,     # resident weights/misc
            tc.tile_pool(name="bigw", bufs=2) as bigw,     # streamed big weights
            tc.tile_pool(name="seq", bufs=1) as seq,       # sequence tensors
            tc.tile_pool(name="work", bufs=2) as work,
            tc.tile_pool(name="psA", bufs=2, space="PSUM") as psA,
            tc.tile_pool(name="psB", bufs=2, space="PSUM") as psB,
        ):
            dma = nc.sync.dma_start

            # ---- resident small tensors ----
            Wr_sb = []
            for l in range(2):
                t_ = wres.tile([PB, 2, 4, 16, PB], BF16, tag=f"Wr{l}", name=f"Wrsb{l}")
                dma(out=t_[:], in_=d_Wr[l][:])
                Wr_sb.append(t_)
            bias_sb = [wres.tile([PB, 2, 16], F32, tag=f"bias{l}", name=f"bias{l}") for l in range(2)]
            h0_sb = [wres.tile([PB, 2, 4], F32, tag=f"h0{l}", name=f"h0{l}") for l in range(2)]
            c0_sb = [wres.tile([PB, 2, 4], F32, tag=f"c0{l}", name=f"c0{l}") for l in range(2)]
            for l in range(2):
                dma(out=bias_sb[l][:], in_=d_bias[l][:])
                dma(out=h0_sb[l][:], in_=d_h0[l][:])
                dma(out=c0_sb[l][:], in_=d_c0[l][:])
            ab1T_sb = wres.tile([128, KC13], F32, tag="ab1T")
            lb1T_sb = wres.tile([128, KC13], F32, tag="lb1T")
            aW2T_sb = wres.tile([128, KC13], BF16, tag="aW2T")
            lW2T_sb = wres.tile([128, KC13, L], BF16, tag="lW2T")
            lb2b_sb = wres.tile([128, L], F32, tag="lb2b")
            arcsel_sb = wres.tile([128, 2, 2, 128], BF16, tag="arcsel")
            idf_sb = wres.tile([128, 128], F32, tag="idf")
            idb_sb = wres.tile([128, 128], BF16, tag="idb")
            isel_sb = wres.tile([128, 2, RB], BF16, tag="isel")
            ab2b_sb = wres.tile([1, 1], F32, tag="ab2b")
            for t_, d_ in [(ab1T_sb, d_ab1T), (lb1T_sb, d_lb1T), (aW2T_sb, d_aW2T),
                           (lW2T_sb, d_lW2T), (lb2b_sb, d_lb2b), (arcsel_sb, d_arcsel),
                           (idf_sb, d_identf), (idb_sb, d_identb), (isel_sb, d_isel),
                           (ab2b_sb, d_ab2b)]:
                dma(out=t_[:], in_=d_[:])

            # ---- stage A: embedding gather ----
            widx_sb = work.tile([128, 2], I32, tag="widx")
            tidx_sb = work.tile([128, 2], I32, tag="tidx")
            for tcc in range(2):
                dma(out=widx_sb[:, tcc:tcc + 1], in_=d_widx[tcc * 128:(tcc + 1) * 128, :])
                dma(out=tidx_sb[:, tcc:tcc + 1], in_=d_tidx[tcc * 128:(tcc + 1) * 128, :])
            xw_sb = [seq.tile([128, WD], F32, tag=f"xw{i}", name=f"xw{i}") for i in range(2)]
            xt_sb = [seq.tile([128, TD], F32, tag=f"xt{i}", name=f"xt{i}") for i in range(2)]
            for tcc in range(2):
                nc.gpsimd.indirect_dma_start(
                    out=xw_sb[tcc][:], out_offset=None, in_=d_embw[:],
                    in_offset=IndirectOffsetOnAxis(ap=widx_sb[:, tcc:tcc + 1], axis=0))
                nc.gpsimd.indirect_dma_start(
                    out=xt_sb[tcc][:], out_offset=None, in_=d_embt[:],
                    in_offset=IndirectOffsetOnAxis(ap=tidx_sb[:, tcc:tcc + 1], axis=0))

            # ---- stage B: xT [100, 4, 256] bf16 ----
            xT = seq.tile([PB, 4, T], BF16, tag="xT")
            for tcc in range(2):
                for c in range(4):
                    src = (xw_sb[tcc][:, c * PB:(c + 1) * PB] if c < 3
                           else xt_sb[tcc][:])
                    pt = psA.tile([PB, 128], F32, tag="ps", name="ptr")
                    nc.tensor.transpose(out=pt[:], in_=src, identity=idf_sb[:])
                    nc.scalar.activation(
                        out=xT[:, c, tcc * 128:(tcc + 1) * 128], in_=pt[:], func=AF.Copy)

            # ---- LSTM helper ----
            def input_side(l, xT_t, kin):
                """Returns XT tiles [100, 16, 256] f32 for dirs 0,1 of layer l."""
                XT_t = []
                for d in range(2):
                    Wi_sb = bigw.tile([PB, kin, 16, PB], BF16, tag="bigw",
                                      name=f"Wi{l}{d}")
                    dma(out=Wi_sb[:], in_=d_Wi[l][:, d])
                    Xt = seq.tile([PB, 16, T], BF16, tag=f"XT{d}", name=f"XT{l}{d}")
                    for mc in range(16):
                        px = psA.tile([PB, T], F32, tag="ps", name="px")
                        for kc in range(kin):
                            nc.tensor.matmul(
                                out=px[:], lhsT=Wi_sb[:, kc, mc, :],
                                rhs=xT_t[:, kc, :],
                                start=(kc == 0), stop=(kc == kin - 1))
                        nc.scalar.activation(
                            out=Xt[:, mc, :], in_=px[:], func=AF.Identity,
                            bias=bias_sb[l][:, d, mc:mc + 1], scale=1.0)
                    XT_t.append(Xt)
                return XT_t

            def recur(l, XT_t):
                """Runs fwd+bwd recurrences; returns (hfT, hbT) [100, 4, 257] bf16."""
                outs = []
                for d in range(2):
                    hT = seq.tile([PB, 4, T + 1], BF16, tag=f"hT{l}{d}")
                    c_t = work.tile([PB, 4], F32, tag=f"c{l}{d}")
                    # init: fwd h at slot 0, bwd h at slot 256 (true-time storage)
                    init_s = 0 if d == 0 else T
                    nc.vector.tensor_copy(out=hT[:, :, init_s:init_s + 1],
                                          in_=h0_sb[l][:, d, :, None])
                    nc.vector.tensor_copy(out=c_t[:], in_=c0_sb[l][:, d, :, None])
                    Xt = XT_t[d]

                    def step(i):
                        # true time tt; fwd reads h at slot tt, writes tt+1
                        # bwd reads h at slot tt+1, writes tt
                        if d == 0:
                            tt, rd_off, wr_off = i, 0, 1
                        else:
                            tt, rd_off, wr_off = 255 - i, 1, 0
                        pg = psB.tile([PB, 16], F32, tag="psb", name="pg")
                        for mc in range(16):
                            for kc in range(4):
                                nc.tensor.matmul(
                                    out=pg[:, mc:mc + 1],
                                    lhsT=Wr_sb[l][:, d, kc, mc, :],
                                    rhs=hT[:, kc, ds(tt + rd_off, 1)],
                                    start=(kc == 0), stop=(kc == 3))
                        gs = work.tile([PB, 16], F32, tag="gs")
                        nc.vector.tensor_tensor(
                            out=gs[:], in0=pg[:], in1=Xt[:, :, ds(tt, 1)], op=OP.add)
                        sg = work.tile([PB, 12], F32, tag="sg")
                        tg = work.tile([PB, 4], F32, tag="tg")
                        nc.scalar.activation(out=sg[:], in_=gs[:, 0:12], func=AF.Sigmoid)
                        nc.scalar.activation(out=tg[:], in_=gs[:, 12:16], func=AF.Tanh)
                        t2 = work.tile([PB, 4], F32, tag="t2")
                        t3 = work.tile([PB, 4], F32, tag="t3")
                        nc.vector.tensor_tensor(out=t2[:], in0=sg[:, 0:4], in1=tg[:], op=OP.mult)
                        nc.vector.tensor_tensor(out=t3[:], in0=sg[:, 4:8], in1=c_t[:], op=OP.mult)
                        nc.vector.tensor_tensor(out=c_t[:], in0=t2[:], in1=t3[:], op=OP.add)
                        tc2 = work.tile([PB, 4], F32, tag="tc2")
                        nc.scalar.activation(out=tc2[:], in_=c_t[:], func=AF.Tanh)
                        nc.vector.tensor_tensor(
                            out=hT[:, :, ds(tt + wr_off, 1)], in0=sg[:, 8:12],
                            in1=tc2[:], op=OP.mult)

                    if unroll <= 1:
                        with tc.For_i(0, T, name=f"lstm{l}{d}") as i:
                            step(i)
                    else:
                        tc.For_i_unrolled(0, T, 1, step, max_unroll=unroll)
                    outs.append(hT)
                return outs

            # ---- L0 ----
            XT0 = input_side(0, xT, 4)
            h0f, h0b = recur(0, XT0)

            # ---- x1T assembly + L1 ----
            x1T = seq.tile([PB, 8, T], BF16, tag="x1T")
            nc.vector.tensor_copy(out=x1T[:, 0:4, :], in_=h0f[:, :, 1:T + 1])
            nc.vector.tensor_copy(out=x1T[:, 4:8, :], in_=h0b[:, :, 0:T])
            XT1 = input_side(1, x1T, 8)
            h1f, h1b = recur(1, XT1)

            # ---- hvT [100, 8, 256] bf16 and hv_n 2x[128, 800] bf16 ----
            hvT = seq.tile([PB, 8, T], BF16, tag="hvT")
            nc.vector.tensor_copy(out=hvT[:, 0:4, :], in_=h1f[:, :, 1:T + 1])
            nc.vector.tensor_copy(out=hvT[:, 4:8, :], in_=h1b[:, :, 0:T])
            hv_n = [seq.tile([128, BI], BF16, tag=f"hvn{i}", name=f"hvn{i}") for i in range(2)]
            for tcc in range(2):
                for c in range(8):
                    pt = psA.tile([128, PB], BF16, tag="ps", name="ptr2")
                    nc.tensor.transpose(
                        out=pt[:], in_=hvT[:, c, tcc * 128:(tcc + 1) * 128],
                        identity=idb_sb[0:PB, 0:PB])
                    nc.scalar.activation(
                        out=hv_n[tcc][:, c * PB:(c + 1) * PB], in_=pt[:], func=AF.Copy)

            # ---- AdbT [128, 13, 256] f32 (Ad + ab1) ----
            aW1d_sb = bigw.tile([PB, 8, KC13, 128], BF16, tag="bigw")
            dma(out=aW1d_sb[:], in_=d_aW1d[:])
            AdbT = seq.tile([128, KC13, T], F32, tag="AdbT")
            for mc in range(KC13):
                pa = psA.tile([128, T], F32, tag="ps", name="pa")
                for kc in range(8):
                    nc.tensor.matmul(out=pa[:], lhsT=aW1d_sb[:, kc, mc, :],
                                     rhs=hvT[:, kc, :], start=(kc == 0), stop=(kc == 7))
                nc.scalar.activation(out=AdbT[:, mc, :], in_=pa[:], func=AF.Identity,
                                     bias=ab1T_sb[:, mc:mc + 1], scale=1.0)

            # ---- hv_my [32, 800] -> hv_myT [100, 8, 32] -> AhT_my [128, 13, 32] ----
            hv_my = work.tile([RB, BI], BF16, tag="hvmy")
            for nh in range(2):
                ps = psB.tile([RB, 400], F32, tag="psb", name="psel")
                for tcc in range(2):
                    nc.tensor.matmul(out=ps[:], lhsT=isel_sb[:, tcc, :],
                                     rhs=hv_n[tcc][:, nh * 400:(nh + 1) * 400],
                                     start=(tcc == 0), stop=(tcc == 1))
                nc.scalar.activation(out=hv_my[:, nh * 400:(nh + 1) * 400],
                                     in_=ps[:], func=AF.Copy)
            hv_myT = work.tile([PB, 8, RB], BF16, tag="hvmyT")
            for c in range(8):
                pt = psA.tile([PB, RB], BF16, tag="ps", name="ptr3")
                nc.tensor.transpose(out=pt[:], in_=hv_my[:, c * PB:(c + 1) * PB],
                                    identity=idb_sb[0:RB, 0:RB])
                nc.scalar.activation(out=hv_myT[:, c, :], in_=pt[:], func=AF.Copy)
            aW1h_sb = bigw.tile([PB, 8, KC13, 128], BF16, tag="bigw")
            dma(out=aW1h_sb[:], in_=d_aW1h[:])
            AhT_my = work.tile([128, KC13, RB], F32, tag="AhTmy")
            for mc in range(KC13):
                pa = psB.tile([128, RB], F32, tag="psb", name="pah")
                for kc in range(8):
                    nc.tensor.matmul(out=pa[:], lhsT=aW1h_sb[:, kc, mc, :],
                                     rhs=hv_myT[:, kc, :], start=(kc == 0), stop=(kc == 7))
                nc.scalar.activation(out=AhT_my[:, mc, :], in_=pa[:], func=AF.Copy)

            # ---- arc grid: 32 rows x 256 cols ----
            for i in range(RB):
                pr = psB.tile([1, T], F32, tag="psb", name="prow")
                for mc in range(KC13):
                    th = work.tile([128, T], BF16, tag="th")
                    nc.scalar.activation(out=th[:], in_=AdbT[:, mc, :], func=AF.Tanh,
                                         bias=AhT_my[:, mc, i:i + 1], scale=1.0)
                    kk = 128 if mc < KC13 - 1 else 64
                    nc.tensor.matmul(out=pr[:], lhsT=aW2T_sb[0:kk, mc:mc + 1],
                                     rhs=th[0:kk, :], start=(mc == 0),
                                     stop=(mc == KC13 - 1))
                srow = work.tile([1, T], F32, tag="srow", name="srow")
                nc.scalar.activation(out=srow[:], in_=pr[:],
                                     func=AF.Identity, bias=ab2b_sb[:, 0:1], scale=1.0)
                mrowA = work.tile([1, T], F32, tag="mrowA", name="mrowA")
                mrowB = work.tile([1, T], F32, tag="mrowB", name="mrowB")
                dma(out=mrowA[:], in_=d_maskA[i:i + 1, :])
                dma(out=mrowB[:], in_=d_maskB[i:i + 1, :])
                sr2 = work.tile([1, T], F32, tag="sr2", name="sr2")
                nc.vector.tensor_tensor(out=sr2[:], in0=srow[:], in1=mrowA[:], op=OP.mult)
                sr3 = work.tile([1, T], F32, tag="sr3", name="sr3")
                nc.vector.tensor_tensor(out=sr3[:], in0=sr2[:], in1=mrowB[:], op=OP.add)
                dma(out=o_scores[i:i + 1, :], in_=sr3[:])

            # ---- labels (replicated) ----
            hv_arcs = [seq.tile([128, BI], BF16, tag=f"hva{i}", name=f"hva{i}") for i in range(2)]
            for tcc in range(2):
                for nh in range(2):
                    ps = psB.tile([128, 400], F32, tag="psb", name="psel2")
                    for tcp in range(2):
                        nc.tensor.matmul(out=ps[:], lhsT=arcsel_sb[:, tcp, tcc, :],
                                         rhs=hv_n[tcp][:, nh * 400:(nh + 1) * 400],
                                         start=(tcp == 0), stop=(tcp == 1))
                    nc.scalar.activation(out=hv_arcs[tcc][:, nh * 400:(nh + 1) * 400],
                                         in_=ps[:], func=AF.Copy)
            hv_arcsT = seq.tile([PB, 8, T], BF16, tag="hvaT")
            for tcc in range(2):
                for c in range(8):
                    pt = psA.tile([PB, 128], BF16, tag="ps", name="ptr4")
                    nc.tensor.transpose(out=pt[:], in_=hv_arcs[tcc][:, c * PB:(c + 1) * PB],
                                        identity=idb_sb[:])
                    nc.scalar.activation(
                        out=hv_arcsT[:, c, tcc * 128:(tcc + 1) * 128], in_=pt[:], func=AF.Copy)
            lW1h_sb = bigw.tile([PB, 8, KC13, 128], BF16, tag="bigw")
            dma(out=lW1h_sb[:], in_=d_lW1h[:])
            lW1d_sb = bigw.tile([PB, 8, KC13, 128], BF16, tag="bigw")
            dma(out=lW1d_sb[:], in_=d_lW1d[:])
            lhT = seq.tile([128, KC13, T], BF16, tag="lhT")
            for mc in range(KC13):
                pl = psA.tile([128, T], F32, tag="ps", name="pl")
                for kc in range(8):
                    nc.tensor.matmul(out=pl[:], lhsT=lW1h_sb[:, kc, mc, :],
                                     rhs=hvT[:, kc, :], start=(kc == 0), stop=False)
                for kc in range(8):
                    nc.tensor.matmul(out=pl[:], lhsT=lW1d_sb[:, kc, mc, :],
                                     rhs=hv_arcsT[:, kc, :], start=False, stop=(kc == 7))
                nc.scalar.activation(out=lhT[:, mc, :], in_=pl[:], func=AF.Tanh,
                                     bias=lb1T_sb[:, mc:mc + 1], scale=1.0)
            for tcc in range(2):
                plab = psB.tile([128, L], F32, tag="psb", name="plab")
                for mc in range(KC13):
                    nc.tensor.matmul(out=plab[:],
                                     lhsT=lhT[:, mc, tcc * 128:(tcc + 1) * 128],
                                     rhs=lW2T_sb[:, mc, :], start=(mc == 0),
                                     stop=(mc == KC13 - 1))
                lab = work.tile([128, L], F32, tag="lab")
                nc.vector.tensor_tensor(out=lab[:], in0=plab[:], in1=lb2b_sb[:], op=OP.add)
                dma(out=o_labels[tcc * 128:(tcc + 1) * 128, :], in_=lab[:])

    nc.finalize()
    return nc


def kernel(**inputs):
    sh, per_core, ab2 = _prep(inputs)
    nc = _build(ab2)
    in_maps = [{**sh, **per_core[c]} for c in range(NC_)]
    res = run_bass_kernel_spmd(nc, in_maps, core_ids=list(range(NC_)))
    scores = np.concatenate([res.results[c]["scores_blk"] for c in range(NC_)], axis=0)
    labels = res.results[0]["labels_o"]
    return scores.astype(np.float32), labels.astype(np.float32)


if __name__ == "__main__":
    import reference
    inputs = {k: np.asarray(v) for k, v in reference.setup_inputs().items()}
    s, lab = kernel(**inputs)
    print("scores", s.shape, "labels", lab.shape)


# revision 15
# speedup vs baseline: 8185.9005x; 8185.9005x over previous
"""Trainium2 Bass kernel for a BiLSTM dependency-parse model.

Computes, on 8 NeuronCores (SPMD, per-core input data differences only):
  x  = concat(emb_word[words], emb_tag[tags])           [256, 400]
  hv = 2-layer BiLSTM(x)                                [256, 800]
  scores[i,j] = aW2 . tanh(Ah[i] + Ad[j] + ab1) (+mask) [256, 256]
  labels      = (tanh(hv@lW1h.T + hv[arcs]@lW1d.T + lb1)) @ lW2.T + lb2

Sharding: the LSTM runs replicated on every core; the [256,256] arc grid is
row-sharded (32 head rows per core) via per-core one-hot selection inputs;
labels are computed replicated (cheap) and taken from core 0.

Layouts (device):
  hidden vectors v[400]   -> [100 part, 4 cols]   (v[100*c+p])
  gate vectors  g[1600]   -> [100 part, 16 cols]  gate order [i,f,o,g]
  sequences     hvT        -> [100, 8, 256]       (BI=800 on partitions)
  MLPIN k=1600            -> [128 part, 13 chunks] (tail chunk 64 valid)
"""
import sys
sys.path.insert(0, "/opt/trn_rl_repo")

import numpy as np
import ml_dtypes

from concourse import bass, bacc, mybir
from concourse.bass import ds, IndirectOffsetOnAxis
from concourse.tile import TileContext
from concourse.bass_utils import run_bass_kernel_spmd

T = 256
V = 50000
NTAGS = 50
L = 40
WD = 300
TD = 100
IN = 400
H = 400
BI = 800
MLPIN = 1600
PB = 100          # partition size for hidden/gate layouts
NC_ = 8           # cores
RB = T // NC_     # 32 score rows per core
KC13 = 13         # MLPIN chunks of 128 (last 64 valid)
F32 = mybir.dt.float32
BF16 = mybir.dt.bfloat16
I32 = mybir.dt.int32
AF = mybir.ActivationFunctionType
OP = mybir.AluOpType
BF = ml_dtypes.bfloat16

# new gate order [i, f, o, g]; offsets into the pytorch [i,f,g,o] row layout
OLD_OFF = [0, 400, 1200, 800]


def _perm_rows(M):
    """[1600, ...] pytorch-gate-ordered -> [16, 100, ...] new-layout blocks."""
    out = np.empty((16, 100) + M.shape[1:], M.dtype)
    for mc in range(16):
        gate, c = mc // 4, mc % 4
        r0 = OLD_OFF[gate] + c * 100
        out[mc] = M[r0:r0 + 100]
    return out


def _w_stat(W):
    """W [1600, KIN*100] -> lhsT stat blocks [100(k), KIN, 16, 100(m)] (part-major)."""
    kin = W.shape[1] // PB
    Wp = _perm_rows(W)                                   # [16, 100m, kin*100]
    S = Wp.reshape(16, 100, kin, 100)                    # [mc, m, kc, k]
    return np.ascontiguousarray(S.transpose(3, 2, 0, 1))  # [k, kc, mc, m]


def _bi_stat(W):
    """W [1600, 800] -> lhsT blocks [100(k), 8, 13, 128(m)], m-padded with 0."""
    Wp = np.zeros((KC13 * 128, BI), np.float32)
    Wp[:MLPIN] = W
    S = Wp.reshape(KC13, 128, 8, 100)                    # [mc, m, kc, k]
    return np.ascontiguousarray(S.transpose(3, 2, 0, 1))  # [k, kc, mc, m]


def _k13(v):
    """v [1600] -> [128, 13] (padded)."""
    p = np.zeros(KC13 * 128, np.float32)
    p[:MLPIN] = v
    return np.ascontiguousarray(p.reshape(KC13, 128).T)


def _hperm(v):
    """v [400] -> [100, 4]."""
    return np.ascontiguousarray(v.reshape(4, 100).T)


def _prep(inputs):
    """Host-side prep: returns (shared_map, per_core_maps)."""
    g = {k: np.asarray(v) for k, v in inputs.items()}
    f32 = lambda a: np.ascontiguousarray(a, np.float32)
    bf = lambda a: np.ascontiguousarray(a).astype(np.float32).astype(BF)

    sh = {}
    sh["emb_word"] = f32(g["emb_word"])
    sh["emb_tag"] = f32(g["emb_tag"])
    sh["widx"] = np.ascontiguousarray(g["words_tensor"].astype(np.int32).reshape(T, 1))
    sh["tidx"] = np.ascontiguousarray(g["tags_tensor"].astype(np.int32).reshape(T, 1))

    # LSTM weights: Wi_l [100, 2, KIN, 16, 100], Wr_l [100, 2, 4, 16, 100]
    for l, (Wih, Whh, bih, bhh) in enumerate(
        [(g["Wih0"], g["Whh0"], g["bih0"], g["bhh0"]),
         (g["Wih1"], g["Whh1"], g["bih1"], g["bhh1"])]):
        wi = np.stack([_w_stat(np.asarray(Wih[d], np.float32)) for d in range(2)], 1)
        wr = np.stack([_w_stat(np.asarray(Whh[d], np.float32)) for d in range(2)], 1)
        bs = np.stack(
            [_perm_rows(np.asarray(bih[d] + bhh[d], np.float32)).T for d in range(2)], 1)
        sh[f"Wi{l}"] = bf(wi)
        sh[f"Wr{l}"] = bf(wr)
        sh[f"bias{l}"] = f32(bs)                          # [100, 2, 16]
    h0, c0 = np.asarray(g["h0"], np.float32), np.asarray(g["c0"], np.float32)
    sh["h00"] = f32(np.stack([_hperm(h0[0]), _hperm(h0[1])], 1))   # [100, 2, 4]
    sh["c00"] = f32(np.stack([_hperm(c0[0]), _hperm(c0[1])], 1))
    sh["h01"] = f32(np.stack([_hperm(h0[2]), _hperm(h0[3])], 1))
    sh["c01"] = f32(np.stack([_hperm(c0[2]), _hperm(c0[3])], 1))

    aW1 = np.asarray(g["aW1"], np.float32)
    lW1 = np.asarray(g["lW1"], np.float32)
    sh["aW1h_s"] = bf(_bi_stat(aW1[:, :BI]))
    sh["aW1d_s"] = bf(_bi_stat(aW1[:, BI:]))
    sh["lW1h_s"] = bf(_bi_stat(lW1[:, :BI]))
    sh["lW1d_s"] = bf(_bi_stat(lW1[:, BI:]))
    sh["ab1T"] = f32(_k13(np.asarray(g["ab1"], np.float32)))
    sh["lb1T"] = f32(_k13(np.asarray(g["lb1"], np.float32)))
    sh["aW2T"] = bf(_k13(np.asarray(g["aW2"], np.float32)[0]))
    lW2p = np.zeros((L, KC13 * 128), np.float32)
    lW2p[:, :MLPIN] = np.asarray(g["lW2"], np.float32)
    sh["lW2T"] = bf(lW2p.reshape(L, KC13, 128).transpose(2, 1, 0))  # [128, 13, 40]
    sh["lb2b"] = f32(np.tile(np.asarray(g["lb2"], np.float32), (128, 1)))
    sh["ab2b"] = np.asarray(g["ab2"], np.float32).reshape(1, 1)

    arcs = np.asarray(g["arcs_refdata_tensor"]).astype(np.int64)
    A = np.zeros((T, T), np.float32)
    A[arcs, np.arange(T)] = 1.0                           # A[t', t] = 1 iff arcs[t]==t'
    sh["arcsel"] = bf(A.reshape(2, 128, 2, 128).transpose(1, 0, 2, 3))  # [128,tcp,tc,128]

    sh["ident_f"] = np.eye(128, dtype=np.float32)
    sh["ident_b"] = np.eye(128, dtype=np.float32).astype(BF)

    ab2 = float(np.asarray(g["ab2"], np.float32).reshape(-1)[0])

    per_core = []
    for c in range(NC_):
        m = {}
        I = np.zeros((T, RB), np.float32)
        I[np.arange(c * RB, (c + 1) * RB), np.arange(RB)] = 1.0
        m["isel"] = bf(I.reshape(2, 128, RB).transpose(1, 0, 2))   # [128, 2, 32]
        mA = np.ones((RB, T), np.float32)
        mA[np.arange(RB), np.arange(c * RB, (c + 1) * RB)] = 0.0
        mB = np.zeros((RB, T), np.float32)
        if c == 0:
            mB[0, 0] = 1.0
        m["maskA"], m["maskB"] = mA, mB
        per_core.append(m)
    return sh, per_core, ab2


def _build(ab2: float, unroll: int = 1, nsteps: int = T, narc: int = RB):
    nc = bacc.Bacc(None, target_bir_lowering=False)
    P = lambda n, s, dt: nc.declare_dram_parameter(n, list(s), dt, isOutput=False)

    d_embw = P("emb_word", (V, WD), F32)
    d_embt = P("emb_tag", (NTAGS, TD), F32)
    d_widx = P("widx", (T, 1), I32)
    d_tidx = P("tidx", (T, 1), I32)
    d_Wi = [P("Wi0", (PB, 2, 4, 16, PB), BF16), P("Wi1", (PB, 2, 8, 16, PB), BF16)]
    d_Wr = [P("Wr0", (PB, 2, 4, 16, PB), BF16), P("Wr1", (PB, 2, 4, 16, PB), BF16)]
    d_bias = [P("bias0", (PB, 2, 16), F32), P("bias1", (PB, 2, 16), F32)]
    d_h0 = [P("h00", (PB, 2, 4), F32), P("h01", (PB, 2, 4), F32)]
    d_c0 = [P("c00", (PB, 2, 4), F32), P("c01", (PB, 2, 4), F32)]
    d_aW1h = P("aW1h_s", (PB, 8, KC13, 128), BF16)
    d_aW1d = P("aW1d_s", (PB, 8, KC13, 128), BF16)
    d_lW1h = P("lW1h_s", (PB, 8, KC13, 128), BF16)
    d_lW1d = P("lW1d_s", (PB, 8, KC13, 128), BF16)
    d_ab1T = P("ab1T", (128, KC13), F32)
    d_lb1T = P("lb1T", (128, KC13), F32)
    d_aW2T = P("aW2T", (128, KC13), BF16)
    d_lW2T = P("lW2T", (128, KC13, L), BF16)
    d_lb2b = P("lb2b", (128, L), F32)
    d_ab2b = P("ab2b", (1, 1), F32)
    d_arcsel = P("arcsel", (128, 2, 2, 128), BF16)
    d_identf = P("ident_f", (128, 128), F32)
    d_identb = P("ident_b", (128, 128), BF16)
    d_isel = P("isel", (128, 2, RB), BF16)
    d_maskA = P("maskA", (RB, T), F32)
    d_maskB = P("maskB", (RB, T), F32)

    o_scores = nc.declare_dram_parameter("scores_blk", [RB, T], F32, isOutput=True)
    o_labels = nc.declare_dram_parameter("labels_o", [T, L], F32, isOutput=True)

    with TileContext(nc) as tc:
        with (
            tc.tile_pool(name="wres", bufs=1) as wres,     # resident weights/misc
            tc.tile_pool(name="bigw", bufs=2) as bigw,     # streamed big weights
            tc.tile_pool(name="seq", bufs=1) as seq,       # sequence tensors
            tc.tile_pool(name="work", bufs=2) as work,
            tc.tile_pool(name="psA", bufs=2, space="PSUM") as psA,
            tc.tile_pool(name="psB", bufs=2, space="PSUM") as psB,
        ):
            dma = nc.sync.dma_start

            # ---- resident small tensors ----
            Wr_sb = []
            for l in range(2):
                t_ = wres.tile([PB, 2, 4, 16, PB], BF16, tag=f"Wr{l}", name=f"Wrsb{l}")
                dma(out=t_[:], in_=d_Wr[l][:])
                Wr_sb.append(t_)
            bias_sb = [wres.tile([PB, 2, 16], F32, tag=f"bias{l}", name=f"bias{l}") for l in range(2)]
            h0_sb = [wres.tile([PB, 2, 4], F32, tag=f"h0{l}", name=f"h0{l}") for l in range(2)]
            c0_sb = [wres.tile([PB, 2, 4], F32, tag=f"c0{l}", name=f"c0{l}") for l in range(2)]
            for l in range(2):
                dma(out=bias_sb[l][:], in_=d_bias[l][:])
                dma(out=h0_sb[l][:], in_=d_h0[l][:])
                dma(out=c0_sb[l][:], in_=d_c0[l][:])
            ab1T_sb = wres.tile([128, KC13], F32, tag="ab1T")
            lb1T_sb = wres.tile([128, KC13], F32, tag="lb1T")
            aW2T_sb = wres.tile([128, KC13], BF16, tag="aW2T")
            lW2T_sb = wres.tile([128, KC13, L], BF16, tag="lW2T")
            lb2b_sb = wres.tile([128, L], F32, tag="lb2b")
            arcsel_sb = wres.tile([128, 2, 2, 128], BF16, tag="arcsel")
            idf_sb = wres.tile([128, 128], F32, tag="idf")
            idb_sb = wres.tile([128, 128], BF16, tag="idb")
            isel_sb = wres.tile([128, 2, RB], BF16, tag="isel")
            ab2b_sb = wres.tile([1, 1], F32, tag="ab2b")
            for t_, d_ in [(ab1T_sb, d_ab1T), (lb1T_sb, d_lb1T), (aW2T_sb, d_aW2T),
                           (lW2T_sb, d_lW2T), (lb2b_sb, d_lb2b), (arcsel_sb, d_arcsel),
                           (idf_sb, d_identf), (idb_sb, d_identb), (isel_sb, d_isel),
                           (ab2b_sb, d_ab2b)]:
                dma(out=t_[:], in_=d_[:])

            # ---- stage A: embedding gather ----
            widx_sb = work.tile([128, 2], I32, tag="widx")
            tidx_sb = work.tile([128, 2], I32, tag="tidx")
            for tcc in range(2):
                dma(out=widx_sb[:, tcc:tcc + 1], in_=d_widx[tcc * 128:(tcc + 1) * 128, :])
                dma(out=tidx_sb[:, tcc:tcc + 1], in_=d_tidx[tcc * 128:(tcc + 1) * 128, :])
            xw_sb = [seq.tile([128, WD], F32, tag=f"xw{i}", name=f"xw{i}") for i in range(2)]
            xt_sb = [seq.tile([128, TD], F32, tag=f"xt{i}", name=f"xt{i}") for i in range(2)]
            for tcc in range(2):
                nc.gpsimd.indirect_dma_start(
                    out=xw_sb[tcc][:], out_offset=None, in_=d_embw[:],
                    in_offset=IndirectOffsetOnAxis(ap=widx_sb[:, tcc:tcc + 1], axis=0))
                nc.gpsimd.indirect_dma_start(
                    out=xt_sb[tcc][:], out_offset=None, in_=d_embt[:],
                    in_offset=IndirectOffsetOnAxis(ap=tidx_sb[:, tcc:tcc + 1], axis=0))

            # ---- stage B: xT [100, 4, 256] bf16 ----
            xT = seq.tile([PB, 4, T], BF16, tag="xT")
            for tcc in range(2):
                for c in range(4):
                    src = (xw_sb[tcc][:, c * PB:(c + 1) * PB] if c < 3
                           else xt_sb[tcc][:])
                    pt = psA.tile([PB, 128], F32, tag="ps", name="ptr")
                    nc.tensor.transpose(out=pt[:], in_=src, identity=idf_sb[:])
                    nc.scalar.activation(
                        out=xT[:, c, tcc * 128:(tcc + 1) * 128], in_=pt[:], func=AF.Copy)

            # ---- LSTM helper ----
            def input_side(l, xT_t, kin):
                """Returns XT tiles [100, 16, 256] f32 for dirs 0,1 of layer l."""
                XT_t = []
                for d in range(2):
                    Wi_sb = bigw.tile([PB, kin, 16, PB], BF16, tag="bigw",
                                      name=f"Wi{l}{d}")
                    dma(out=Wi_sb[:], in_=d_Wi[l][:, d])
                    Xt = seq.tile([PB, 16, T], BF16, tag=f"XT{d}", name=f"XT{l}{d}")
                    for mc in range(16):
                        px = psA.tile([PB, T], F32, tag="ps", name="px")
                        for kc in range(kin):
                            nc.tensor.matmul(
                                out=px[:], lhsT=Wi_sb[:, kc, mc, :],
                                rhs=xT_t[:, kc, :],
                                start=(kc == 0), stop=(kc == kin - 1))
                        nc.scalar.activation(
                            out=Xt[:, mc, :], in_=px[:], func=AF.Identity,
                            bias=bias_sb[l][:, d, mc:mc + 1], scale=1.0)
                    XT_t.append(Xt)
                return XT_t

            def recur(l, XT_t):
                """Runs fwd+bwd recurrences; returns (hfT, hbT) [100, 4, 257] bf16."""
                outs = []
                steps = []
                for d in range(2):
                    hT = seq.tile([PB, 4, T + 1], BF16, tag=f"hT{l}{d}",
                                  name=f"hT{l}{d}")
                    c_t = work.tile([PB, 4], F32, tag=f"c{l}{d}", name=f"c{l}{d}")
                    # init: fwd h at slot 0, bwd h at slot 256 (true-time storage)
                    init_s = 0 if d == 0 else T
                    nc.vector.tensor_copy(out=hT[:, :, init_s:init_s + 1],
                                          in_=h0_sb[l][:, d, :, None])
                    nc.vector.tensor_copy(out=c_t[:], in_=c0_sb[l][:, d, :, None])
                    Xt = XT_t[d]

                    def step(i, d=d, hT=hT, c_t=c_t, Xt=Xt):
                        # true time tt; fwd reads h at slot tt, writes tt+1
                        # bwd reads h at slot tt+1, writes tt
                        if d == 0:
                            tt, rd_off, wr_off = i, 0, 1
                        else:
                            tt, rd_off, wr_off = 255 - i, 1, 0
                        pg = psB.tile([PB, 16], F32, tag="psb", name="pg")
                        for mc in range(16):
                            for kc in range(4):
                                nc.tensor.matmul(
                                    out=pg[:, mc:mc + 1],
                                    lhsT=Wr_sb[l][:, d, kc, mc, :],
                                    rhs=hT[:, kc, ds(tt + rd_off, 1)],
                                    start=(kc == 0), stop=(kc == 3))
                        gs = work.tile([PB, 16], F32, tag="gs")
                        nc.vector.tensor_tensor(
                            out=gs[:], in0=pg[:], in1=Xt[:, :, ds(tt, 1)], op=OP.add)
                        sg = work.tile([PB, 12], F32, tag="sg")
                        tg = work.tile([PB, 4], F32, tag="tg")
                        nc.scalar.activation(out=sg[:], in_=gs[:, 0:12], func=AF.Sigmoid)
                        nc.scalar.activation(out=tg[:], in_=gs[:, 12:16], func=AF.Tanh)
                        t2 = work.tile([PB, 4], F32, tag="t2")
                        t3 = work.tile([PB, 4], F32, tag="t3")
                        nc.vector.tensor_tensor(out=t2[:], in0=sg[:, 0:4], in1=tg[:], op=OP.mult)
                        nc.vector.tensor_tensor(out=t3[:], in0=sg[:, 4:8], in1=c_t[:], op=OP.mult)
                        nc.vector.tensor_tensor(out=c_t[:], in0=t2[:], in1=t3[:], op=OP.add)
                        tc2 = work.tile([PB, 4], F32, tag="tc2")
                        nc.scalar.activation(out=tc2[:], in_=c_t[:], func=AF.Tanh)
                        nc.vector.tensor_tensor(
                            out=hT[:, :, ds(tt + wr_off, 1)], in0=sg[:, 8:12],
                            in1=tc2[:], op=OP.mult)

                    steps.append(step)
                    outs.append(hT)

                def both(i):
                    steps[0](i)
                    steps[1](i)

                if unroll <= 1:
                    with tc.For_i(0, nsteps, name=f"lstm{l}") as i:
                        both(i)
                else:
                    tc.For_i_unrolled(0, nsteps, 1, both, max_unroll=unroll)
                return outs

            # ---- L0 ----
            XT0 = input_side(0, xT, 4)
            h0f, h0b = recur(0, XT0)

            # ---- x1T assembly + L1 ----
            x1T = seq.tile([PB, 8, T], BF16, tag="x1T")
            nc.vector.tensor_copy(out=x1T[:, 0:4, :], in_=h0f[:, :, 1:T + 1])
            nc.vector.tensor_copy(out=x1T[:, 4:8, :], in_=h0b[:, :, 0:T])
            XT1 = input_side(1, x1T, 8)
            h1f, h1b = recur(1, XT1)

            # ---- hvT [100, 8, 256] bf16 and hv_n 2x[128, 800] bf16 ----
            hvT = seq.tile([PB, 8, T], BF16, tag="hvT")
            nc.vector.tensor_copy(out=hvT[:, 0:4, :], in_=h1f[:, :, 1:T + 1])
            nc.vector.tensor_copy(out=hvT[:, 4:8, :], in_=h1b[:, :, 0:T])
            hv_n = [seq.tile([128, BI], BF16, tag=f"hvn{i}", name=f"hvn{i}") for i in range(2)]
            for tcc in range(2):
                for c in range(8):
                    pt = psA.tile([128, PB], BF16, tag="ps", name="ptr2")
                    nc.tensor.transpose(
                        out=pt[:], in_=hvT[:, c, tcc * 128:(tcc + 1) * 128],
                        identity=idb_sb[0:PB, 0:PB])
                    nc.scalar.activation(
                        out=hv_n[tcc][:, c * PB:(c + 1) * PB], in_=pt[:], func=AF.Copy)

            # ---- AdbT [128, 13, 256] f32 (Ad + ab1) ----
            aW1d_sb = bigw.tile([PB, 8, KC13, 128], BF16, tag="bigw")
            dma(out=aW1d_sb[:], in_=d_aW1d[:])
            AdbT = seq.tile([128, KC13, T], F32, tag="AdbT")
            for mc in range(KC13):
                pa = psA.tile([128, T], F32, tag="ps", name="pa")
                for kc in range(8):
                    nc.tensor.matmul(out=pa[:], lhsT=aW1d_sb[:, kc, mc, :],
                                     rhs=hvT[:, kc, :], start=(kc == 0), stop=(kc == 7))
                nc.scalar.activation(out=AdbT[:, mc, :], in_=pa[:], func=AF.Identity,
                                     bias=ab1T_sb[:, mc:mc + 1], scale=1.0)

            # ---- hv_my [32, 800] -> hv_myT [100, 8, 32] -> AhT_my [128, 13, 32] ----
            hv_my = work.tile([RB, BI], BF16, tag="hvmy")
            for nh in range(2):
                ps = psB.tile([RB, 400], F32, tag="psb", name="psel")
                for tcc in range(2):
                    nc.tensor.matmul(out=ps[:], lhsT=isel_sb[:, tcc, :],
                                     rhs=hv_n[tcc][:, nh * 400:(nh + 1) * 400],
                                     start=(tcc == 0), stop=(tcc == 1))
                nc.scalar.activation(out=hv_my[:, nh * 400:(nh + 1) * 400],
                                     in_=ps[:], func=AF.Copy)
            hv_myT = work.tile([PB, 8, RB], BF16, tag="hvmyT")
            for c in range(8):
                pt = psA.tile([PB, RB], BF16, tag="ps", name="ptr3")
                nc.tensor.transpose(out=pt[:], in_=hv_my[:, c * PB:(c + 1) * PB],
                                    identity=idb_sb[0:RB, 0:RB])
                nc.scalar.activation(out=hv_myT[:, c, :], in_=pt[:], func=AF.Copy)
            aW1h_sb = bigw.tile([PB, 8, KC13, 128], BF16, tag="bigw")
            dma(out=aW1h_sb[:], in_=d_aW1h[:])
            AhT_my = work.tile([128, KC13, RB], F32, tag="AhTmy")
            for mc in range(KC13):
                pa = psB.tile([128, RB], F32, tag="psb", name="pah")
                for kc in range(8):
                    nc.tensor.matmul(out=pa[:], lhsT=aW1h_sb[:, kc, mc, :],
                                     rhs=hv_myT[:, kc, :], start=(kc == 0), stop=(kc == 7))
                nc.scalar.activation(out=AhT_my[:, mc, :], in_=pa[:], func=AF.Copy)

            # ---- arc grid: 32 rows x 256 cols ----
            for i in range(narc):
                pr = psB.tile([1, T], F32, tag="psb", name="prow")
                for mc in range(KC13):
                    th = work.tile([128, T], BF16, tag="th")
                    nc.scalar.activation(out=th[:], in_=AdbT[:, mc, :], func=AF.Tanh,
                                         bias=AhT_my[:, mc, i:i + 1], scale=1.0)
                    kk = 128 if mc < KC13 - 1 else 64
                    nc.tensor.matmul(out=pr[:], lhsT=aW2T_sb[0:kk, mc:mc + 1],
                                     rhs=th[0:kk, :], start=(mc == 0),
                                     stop=(mc == KC13 - 1))
                srow = work.tile([1, T], F32, tag="srow", name="srow")
                nc.scalar.activation(out=srow[:], in_=pr[:],
                                     func=AF.Identity, bias=ab2b_sb[:, 0:1], scale=1.0)
                mrowA = work.tile([1, T], F32, tag="mrowA", name="mrowA")
                mrowB = work.tile([1, T], F32, tag="mrowB", name="mrowB")
                dma(out=mrowA[:], in_=d_maskA[i:i + 1, :])
                dma(out=mrowB[:], in_=d_maskB[i:i + 1, :])
                sr2 = work.tile([1, T], F32, tag="sr2", name="sr2")
                nc.vector.tensor_tensor(out=sr2[:], in0=srow[:], in1=mrowA[:], op=OP.mult)
                sr3 = work.tile([1, T], F32, tag="sr3", name="sr3")
                nc.vector.tensor_tensor(out=sr3[:], in0=sr2[:], in1=mrowB[:], op=OP.add)
                dma(out=o_scores[i:i + 1, :], in_=sr3[:])

            # ---- labels (replicated) ----
            hv_arcs = [seq.tile([128, BI], BF16, tag=f"hva{i}", name=f"hva{i}") for i in range(2)]
            for tcc in range(2):
                for nh in range(2):
                    ps = psB.tile([128, 400], F32, tag="psb", name="psel2")
                    for tcp in range(2):
                        nc.tensor.matmul(out=ps[:], lhsT=arcsel_sb[:, tcp, tcc, :],
                                         rhs=hv_n[tcp][:, nh * 400:(nh + 1) * 400],
                                         start=(tcp == 0), stop=(tcp == 1))
                    nc.scalar.activation(out=hv_arcs[tcc][:, nh * 400:(nh + 1) * 400],
                                         in_=ps[:], func=AF.Copy)
            hv_arcsT = seq.tile([PB, 8, T], BF16, tag="hvaT")
            for tcc in range(2):
                for c in range(8):
                    pt = psA.tile([PB, 128], BF16, tag="ps", name="ptr4")
                    nc.tensor.transpose(out=pt[:], in_=hv_arcs[tcc][:, c * PB:(c + 1) * PB],
                                        identity=idb_sb[:])
                    nc.scalar.activation(
                        out=hv_arcsT[:, c, tcc * 128:(tcc + 1) * 128], in_=pt[:], func=AF.Copy)
            lW1h_sb = bigw.tile([PB, 8, KC13, 128], BF16, tag="bigw")
            dma(out=lW1h_sb[:], in_=d_lW1h[:])
            lW1d_sb = bigw.tile([PB, 8, KC13, 128], BF16, tag="bigw")
            dma(out=lW1d_sb[:], in_=d_lW1d[:])
            lhT = seq.tile([128, KC13, T], BF16, tag="lhT")
            for mc in range(KC13):
                pl = psA.tile([128, T], F32, tag="ps", name="pl")
                for kc in range(8):
                    nc.tensor.matmul(out=pl[:], lhsT=lW1h_sb[:, kc, mc, :],
                                     rhs=hvT[:, kc, :], start=(kc == 0), stop=False)
                for kc in range(8):
                    nc.tensor.matmul(out=pl[:], lhsT=lW1d_sb[:, kc, mc, :],
                                     rhs=hv_arcsT[:, kc, :], start=False, stop=(kc == 7))
                nc.scalar.activation(out=lhT[:, mc, :], in_=pl[:], func=AF.Tanh,
                                     bias=lb1T_sb[:, mc:mc + 1], scale=1.0)
            for tcc in range(2):
                plab = psB.tile([128, L], F32, tag="psb", name="plab")
                for mc in range(KC13):
                    nc.tensor.matmul(out=plab[:],
                                     lhsT=lhT[:, mc, tcc * 128:(tcc + 1) * 128],
                                     rhs=lW2T_sb[:, mc, :], start=(mc == 0),
                                     stop=(mc == KC13 - 1))
                lab = work.tile([128, L], F32, tag="lab")
                nc.vector.tensor_tensor(out=lab[:], in0=plab[:], in1=lb2b_sb[:], op=OP.add)
                dma(out=o_labels[tcc * 128:(tcc + 1) * 128, :], in_=lab[:])

    nc.finalize()
    return nc


def kernel(**inputs):
    sh, per_core, ab2 = _prep(inputs)
    nc = _build(ab2, unroll=4)
    in_maps = [{**sh, **per_core[c]} for c in range(NC_)]
    res = run_bass_kernel_spmd(nc, in_maps, core_ids=list(range(NC_)))
    scores = np.concatenate([res.results[c]["scores_blk"] for c in range(NC_)], axis=0)
    labels = res.results[0]["labels_o"]
    return scores.astype(np.float32), labels.astype(np.float32)


if __name__ == "__main__":
    import reference
    inputs = {k: np.asarray(v) for k, v in reference.setup_inputs().items()}
    s, lab = kernel(**inputs)
    print("scores", s.shape, "labels", lab.shape)


# revision 18
# speedup vs baseline: 9039.2671x; 1.1042x over previous
"""Trainium2 Bass kernel for a BiLSTM dependency-parse model.

Computes, on 8 NeuronCores (SPMD, per-core input data differences only):
  x  = concat(emb_word[words], emb_tag[tags])           [256, 400]
  hv = 2-layer BiLSTM(x)                                [256, 800]
  scores[i,j] = aW2 . tanh(Ah[i] + Ad[j] + ab1) (+mask) [256, 256]
  labels      = (tanh(hv@lW1h.T + hv[arcs]@lW1d.T + lb1)) @ lW2.T + lb2

Sharding: the LSTM runs replicated on every core; the [256,256] arc grid is
row-sharded (32 head rows per core) via per-core one-hot selection inputs;
labels are computed replicated (cheap) and taken from core 0.

Layouts (device):
  hidden vectors v[400]   -> [100 part, 4 cols]   (v[100*c+p])
  gate vectors  g[1600]   -> [100 part, 16 cols]  gate order [i,f,o,g]
  sequences     hvT        -> [100, 8, 256]       (BI=800 on partitions)
  MLPIN k=1600            -> [128 part, 13 chunks] (tail chunk 64 valid)
"""
import sys
sys.path.insert(0, "/opt/trn_rl_repo")

import numpy as np
import ml_dtypes

from concourse import bass, bacc, mybir
from concourse.bass import ds, IndirectOffsetOnAxis
from concourse.tile import TileContext
from concourse.bass_utils import run_bass_kernel_spmd

T = 256
V = 50000
NTAGS = 50
L = 40
WD = 300
TD = 100
IN = 400
H = 400
BI = 800
MLPIN = 1600
PB = 100          # partition size for hidden/gate layouts
NC_ = 8           # cores
RB = T // NC_     # 32 score rows per core
KC13 = 13         # MLPIN chunks of 128 (last 64 valid)
F32 = mybir.dt.float32
BF16 = mybir.dt.bfloat16
I32 = mybir.dt.int32
AF = mybir.ActivationFunctionType
OP = mybir.AluOpType
BF = ml_dtypes.bfloat16

# new gate order [i, f, o, g]; offsets into the pytorch [i,f,g,o] row layout
OLD_OFF = [0, 400, 1200, 800]


def _perm_rows(M):
    """[1600, ...] pytorch-gate-ordered -> [16, 100, ...] new-layout blocks."""
    out = np.empty((16, 100) + M.shape[1:], M.dtype)
    for mc in range(16):
        gate, c = mc // 4, mc % 4
        r0 = OLD_OFF[gate] + c * 100
        out[mc] = M[r0:r0 + 100]
    return out


def _w_stat(W):
    """W [1600, KIN*100] -> lhsT stat blocks [100(k), KIN, 16, 100(m)] (part-major)."""
    kin = W.shape[1] // PB
    Wp = _perm_rows(W)                                   # [16, 100m, kin*100]
    S = Wp.reshape(16, 100, kin, 100)                    # [mc, m, kc, k]
    return np.ascontiguousarray(S.transpose(3, 2, 0, 1))  # [k, kc, mc, m]


def _bi_stat(W):
    """W [1600, 800] -> lhsT blocks [100(k), 8, 13, 128(m)], m-padded with 0."""
    Wp = np.zeros((KC13 * 128, BI), np.float32)
    Wp[:MLPIN] = W
    S = Wp.reshape(KC13, 128, 8, 100)                    # [mc, m, kc, k]
    return np.ascontiguousarray(S.transpose(3, 2, 0, 1))  # [k, kc, mc, m]


def _k13(v):
    """v [1600] -> [128, 13] (padded)."""
    p = np.zeros(KC13 * 128, np.float32)
    p[:MLPIN] = v
    return np.ascontiguousarray(p.reshape(KC13, 128).T)


def _hperm(v):
    """v [400] -> [100, 4]."""
    return np.ascontiguousarray(v.reshape(4, 100).T)


def _prep(inputs):
    """Host-side prep: returns (shared_map, per_core_maps)."""
    g = {k: np.asarray(v) for k, v in inputs.items()}
    f32 = lambda a: np.ascontiguousarray(a, np.float32)
    bf = lambda a: np.ascontiguousarray(a).astype(np.float32).astype(BF)

    sh = {}
    sh["emb_word"] = f32(g["emb_word"])
    sh["emb_tag"] = f32(g["emb_tag"])
    sh["widx"] = np.ascontiguousarray(g["words_tensor"].astype(np.int32).reshape(T, 1))
    sh["tidx"] = np.ascontiguousarray(g["tags_tensor"].astype(np.int32).reshape(T, 1))

    # LSTM weights: Wi_l [100, 2, KIN, 16, 100], Wr_l [100, 2, 4, 16, 100]
    for l, (Wih, Whh, bih, bhh) in enumerate(
        [(g["Wih0"], g["Whh0"], g["bih0"], g["bhh0"]),
         (g["Wih1"], g["Whh1"], g["bih1"], g["bhh1"])]):
        wi = np.stack([_w_stat(np.asarray(Wih[d], np.float32)) for d in range(2)], 1)
        wr = np.stack([_w_stat(np.asarray(Whh[d], np.float32)) for d in range(2)], 1)
        bs = np.stack(
            [_perm_rows(np.asarray(bih[d] + bhh[d], np.float32)).T for d in range(2)], 1)
        sh[f"Wi{l}"] = bf(wi)
        sh[f"Wr{l}"] = bf(wr)
        sh[f"bias{l}"] = f32(bs)                          # [100, 2, 16]
    h0, c0 = np.asarray(g["h0"], np.float32), np.asarray(g["c0"], np.float32)
    sh["h00"] = f32(np.stack([_hperm(h0[0]), _hperm(h0[1])], 1))   # [100, 2, 4]
    sh["c00"] = f32(np.stack([_hperm(c0[0]), _hperm(c0[1])], 1))
    sh["h01"] = f32(np.stack([_hperm(h0[2]), _hperm(h0[3])], 1))
    sh["c01"] = f32(np.stack([_hperm(c0[2]), _hperm(c0[3])], 1))

    aW1 = np.asarray(g["aW1"], np.float32)
    lW1 = np.asarray(g["lW1"], np.float32)
    sh["aW1h_s"] = bf(_bi_stat(aW1[:, :BI]))
    sh["aW1d_s"] = bf(_bi_stat(aW1[:, BI:]))
    sh["lW1h_s"] = bf(_bi_stat(lW1[:, :BI]))
    sh["lW1d_s"] = bf(_bi_stat(lW1[:, BI:]))
    sh["ab1T"] = f32(_k13(np.asarray(g["ab1"], np.float32)))
    sh["lb1T"] = f32(_k13(np.asarray(g["lb1"], np.float32)))
    sh["aW2T"] = bf(_k13(np.asarray(g["aW2"], np.float32)[0]))
    lW2p = np.zeros((L, KC13 * 128), np.float32)
    lW2p[:, :MLPIN] = np.asarray(g["lW2"], np.float32)
    sh["lW2T"] = bf(lW2p.reshape(L, KC13, 128).transpose(2, 1, 0))  # [128, 13, 40]
    sh["lb2b"] = f32(np.tile(np.asarray(g["lb2"], np.float32), (128, 1)))
    sh["ab2b"] = np.asarray(g["ab2"], np.float32).reshape(1, 1)

    arcs = np.asarray(g["arcs_refdata_tensor"]).astype(np.int64)
    A = np.zeros((T, T), np.float32)
    A[arcs, np.arange(T)] = 1.0                           # A[t', t] = 1 iff arcs[t]==t'
    sh["arcsel"] = bf(A.reshape(2, 128, 2, 128).transpose(1, 0, 2, 3))  # [128,tcp,tc,128]

    sh["ident_f"] = np.eye(128, dtype=np.float32)
    sh["ident_b"] = np.eye(128, dtype=np.float32).astype(BF)

    ab2 = float(np.asarray(g["ab2"], np.float32).reshape(-1)[0])

    per_core = []
    for c in range(NC_):
        m = {}
        I = np.zeros((T, RB), np.float32)
        I[np.arange(c * RB, (c + 1) * RB), np.arange(RB)] = 1.0
        m["isel"] = bf(I.reshape(2, 128, RB).transpose(1, 0, 2))   # [128, 2, 32]
        mA = np.ones((RB, T), np.float32)
        mA[np.arange(RB), np.arange(c * RB, (c + 1) * RB)] = 0.0
        mB = np.zeros((RB, T), np.float32)
        if c == 0:
            mB[0, 0] = 1.0
        m["maskA"], m["maskB"] = mA, mB
        per_core.append(m)
    return sh, per_core, ab2


def _build(ab2: float, unroll: int = 1, nsteps: int = T, narc: int = RB):
    nc = bacc.Bacc(None, target_bir_lowering=False)
    P = lambda n, s, dt: nc.declare_dram_parameter(n, list(s), dt, isOutput=False)

    d_embw = P("emb_word", (V, WD), F32)
    d_embt = P("emb_tag", (NTAGS, TD), F32)
    d_widx = P("widx", (T, 1), I32)
    d_tidx = P("tidx", (T, 1), I32)
    d_Wi = [P("Wi0", (PB, 2, 4, 16, PB), BF16), P("Wi1", (PB, 2, 8, 16, PB), BF16)]
    d_Wr = [P("Wr0", (PB, 2, 4, 16, PB), BF16), P("Wr1", (PB, 2, 4, 16, PB), BF16)]
    d_bias = [P("bias0", (PB, 2, 16), F32), P("bias1", (PB, 2, 16), F32)]
    d_h0 = [P("h00", (PB, 2, 4), F32), P("h01", (PB, 2, 4), F32)]
    d_c0 = [P("c00", (PB, 2, 4), F32), P("c01", (PB, 2, 4), F32)]
    d_aW1h = P("aW1h_s", (PB, 8, KC13, 128), BF16)
    d_aW1d = P("aW1d_s", (PB, 8, KC13, 128), BF16)
    d_lW1h = P("lW1h_s", (PB, 8, KC13, 128), BF16)
    d_lW1d = P("lW1d_s", (PB, 8, KC13, 128), BF16)
    d_ab1T = P("ab1T", (128, KC13), F32)
    d_lb1T = P("lb1T", (128, KC13), F32)
    d_aW2T = P("aW2T", (128, KC13), BF16)
    d_lW2T = P("lW2T", (128, KC13, L), BF16)
    d_lb2b = P("lb2b", (128, L), F32)
    d_ab2b = P("ab2b", (1, 1), F32)
    d_arcsel = P("arcsel", (128, 2, 2, 128), BF16)
    d_identf = P("ident_f", (128, 128), F32)
    d_identb = P("ident_b", (128, 128), BF16)
    d_isel = P("isel", (128, 2, RB), BF16)
    d_maskA = P("maskA", (RB, T), F32)
    d_maskB = P("maskB", (RB, T), F32)

    o_scores = nc.declare_dram_parameter("scores_blk", [RB, T], F32, isOutput=True)
    o_labels = nc.declare_dram_parameter("labels_o", [T, L], F32, isOutput=True)

    with TileContext(nc) as tc:
        with (
            tc.tile_pool(name="wres", bufs=1) as wres,     # resident weights/misc
            tc.tile_pool(name="bigw", bufs=2) as bigw,     # streamed big weights
            tc.tile_pool(name="seq", bufs=1) as seq,       # sequence tensors
            tc.tile_pool(name="work", bufs=4) as work,
            tc.tile_pool(name="psA", bufs=2, space="PSUM") as psA,
            tc.tile_pool(name="psB", bufs=4, space="PSUM") as psB,
        ):
            dma = nc.sync.dma_start

            # ---- resident small tensors ----
            Wr_sb = []
            for l in range(2):
                t_ = wres.tile([PB, 2, 4, 16, PB], BF16, tag=f"Wr{l}", name=f"Wrsb{l}")
                dma(out=t_[:], in_=d_Wr[l][:])
                Wr_sb.append(t_)
            bias_sb = [wres.tile([PB, 2, 16], F32, tag=f"bias{l}", name=f"bias{l}") for l in range(2)]
            h0_sb = [wres.tile([PB, 2, 4], F32, tag=f"h0{l}", name=f"h0{l}") for l in range(2)]
            c0_sb = [wres.tile([PB, 2, 4], F32, tag=f"c0{l}", name=f"c0{l}") for l in range(2)]
            for l in range(2):
                dma(out=bias_sb[l][:], in_=d_bias[l][:])
                dma(out=h0_sb[l][:], in_=d_h0[l][:])
                dma(out=c0_sb[l][:], in_=d_c0[l][:])
            ab1T_sb = wres.tile([128, KC13], F32, tag="ab1T")
            lb1T_sb = wres.tile([128, KC13], F32, tag="lb1T")
            aW2T_sb = wres.tile([128, KC13], BF16, tag="aW2T")
            lW2T_sb = wres.tile([128, KC13, L], BF16, tag="lW2T")
            lb2b_sb = wres.tile([128, L], F32, tag="lb2b")
            arcsel_sb = wres.tile([128, 2, 2, 128], BF16, tag="arcsel")
            idf_sb = wres.tile([128, 128], F32, tag="idf")
            idb_sb = wres.tile([128, 128], BF16, tag="idb")
            isel_sb = wres.tile([128, 2, RB], BF16, tag="isel")
            ab2b_sb = wres.tile([1, 1], F32, tag="ab2b")
            for t_, d_ in [(ab1T_sb, d_ab1T), (lb1T_sb, d_lb1T), (aW2T_sb, d_aW2T),
                           (lW2T_sb, d_lW2T), (lb2b_sb, d_lb2b), (arcsel_sb, d_arcsel),
                           (idf_sb, d_identf), (idb_sb, d_identb), (isel_sb, d_isel),
                           (ab2b_sb, d_ab2b)]:
                dma(out=t_[:], in_=d_[:])

            # ---- stage A: embedding gather ----
            widx_sb = work.tile([128, 2], I32, tag="widx")
            tidx_sb = work.tile([128, 2], I32, tag="tidx")
            for tcc in range(2):
                dma(out=widx_sb[:, tcc:tcc + 1], in_=d_widx[tcc * 128:(tcc + 1) * 128, :])
                dma(out=tidx_sb[:, tcc:tcc + 1], in_=d_tidx[tcc * 128:(tcc + 1) * 128, :])
            xw_sb = [seq.tile([128, WD], F32, tag=f"xw{i}", name=f"xw{i}") for i in range(2)]
            xt_sb = [seq.tile([128, TD], F32, tag=f"xt{i}", name=f"xt{i}") for i in range(2)]
            for tcc in range(2):
                nc.gpsimd.indirect_dma_start(
                    out=xw_sb[tcc][:], out_offset=None, in_=d_embw[:],
                    in_offset=IndirectOffsetOnAxis(ap=widx_sb[:, tcc:tcc + 1], axis=0))
                nc.gpsimd.indirect_dma_start(
                    out=xt_sb[tcc][:], out_offset=None, in_=d_embt[:],
                    in_offset=IndirectOffsetOnAxis(ap=tidx_sb[:, tcc:tcc + 1], axis=0))

            # ---- stage B: xT [100, 4, 256] bf16 ----
            xT = seq.tile([PB, 4, T], BF16, tag="xT")
            for tcc in range(2):
                for c in range(4):
                    src = (xw_sb[tcc][:, c * PB:(c + 1) * PB] if c < 3
                           else xt_sb[tcc][:])
                    pt = psA.tile([PB, 128], F32, tag="ps", name="ptr")
                    nc.tensor.transpose(out=pt[:], in_=src, identity=idf_sb[:])
                    nc.scalar.activation(
                        out=xT[:, c, tcc * 128:(tcc + 1) * 128], in_=pt[:], func=AF.Copy)

            # ---- LSTM helper ----
            def input_side(l, xT_t, kin):
                """Returns XT tiles [100, 16, 256] f32 for dirs 0,1 of layer l."""
                XT_t = []
                for d in range(2):
                    Wi_sb = bigw.tile([PB, kin, 16, PB], BF16, tag="bigw",
                                      name=f"Wi{l}{d}")
                    dma(out=Wi_sb[:], in_=d_Wi[l][:, d])
                    Xt = seq.tile([PB, 16, T], BF16, tag=f"XT{d}", name=f"XT{l}{d}")
                    for mc in range(16):
                        px = psA.tile([PB, T], F32, tag="ps", name="px")
                        for kc in range(kin):
                            nc.tensor.matmul(
                                out=px[:], lhsT=Wi_sb[:, kc, mc, :],
                                rhs=xT_t[:, kc, :],
                                start=(kc == 0), stop=(kc == kin - 1))
                        nc.scalar.activation(
                            out=Xt[:, mc, :], in_=px[:], func=AF.Identity,
                            bias=bias_sb[l][:, d, mc:mc + 1], scale=1.0)
                    XT_t.append(Xt)
                return XT_t

            def recur(l, XT_t):
                """Runs fwd+bwd recurrences; returns (hfT, hbT) [100, 4, 257] bf16."""
                outs = []
                steps = []
                for d in range(2):
                    hT = seq.tile([PB, 4, T + 1], BF16, tag=f"hT{l}{d}",
                                  name=f"hT{l}{d}")
                    c_t = work.tile([PB, 4], F32, tag=f"c{l}{d}", name=f"c{l}{d}")
                    # init: fwd h at slot 0, bwd h at slot 256 (true-time storage)
                    init_s = 0 if d == 0 else T
                    nc.vector.tensor_copy(out=hT[:, :, init_s:init_s + 1],
                                          in_=h0_sb[l][:, d, :, None])
                    nc.vector.tensor_copy(out=c_t[:], in_=c0_sb[l][:, d, :, None])
                    Xt = XT_t[d]

                    def step(i, d=d, hT=hT, c_t=c_t, Xt=Xt):
                        # true time tt; fwd reads h at slot tt, writes tt+1
                        # bwd reads h at slot tt+1, writes tt
                        if d == 0:
                            tt, rd_off, wr_off = i, 0, 1
                        else:
                            tt, rd_off, wr_off = 255 - i, 1, 0
                        pg = psB.tile([PB, 16], F32, tag="psb", name="pg")
                        for mc in range(16):
                            for kc in range(4):
                                nc.tensor.matmul(
                                    out=pg[:, mc:mc + 1],
                                    lhsT=Wr_sb[l][:, d, kc, mc, :],
                                    rhs=hT[:, kc, ds(tt + rd_off, 1)],
                                    start=(kc == 0), stop=(kc == 3))
                        gs = work.tile([PB, 16], F32, tag="gs")
                        nc.vector.tensor_tensor(
                            out=gs[:], in0=pg[:], in1=Xt[:, :, ds(tt, 1)], op=OP.add)
                        sg = work.tile([PB, 12], F32, tag="sg")
                        tg = work.tile([PB, 4], F32, tag="tg")
                        nc.scalar.activation(out=sg[:], in_=gs[:, 0:12], func=AF.Sigmoid)
                        nc.scalar.activation(out=tg[:], in_=gs[:, 12:16], func=AF.Tanh)
                        t2 = work.tile([PB, 4], F32, tag="t2")
                        t3 = work.tile([PB, 4], F32, tag="t3")
                        nc.vector.tensor_tensor(out=t2[:], in0=sg[:, 0:4], in1=tg[:], op=OP.mult)
                        nc.vector.tensor_tensor(out=t3[:], in0=sg[:, 4:8], in1=c_t[:], op=OP.mult)
                        nc.vector.tensor_tensor(out=c_t[:], in0=t2[:], in1=t3[:], op=OP.add)
                        tc2 = work.tile([PB, 4], F32, tag="tc2")
                        nc.scalar.activation(out=tc2[:], in_=c_t[:], func=AF.Tanh)
                        nc.vector.tensor_tensor(
                            out=hT[:, :, ds(tt + wr_off, 1)], in0=sg[:, 8:12],
                            in1=tc2[:], op=OP.mult)

                    steps.append(step)
                    outs.append(hT)

                def both(i):
                    steps[0](i)
                    steps[1](i)

                if unroll <= 1:
                    with tc.For_i(0, nsteps, name=f"lstm{l}") as i:
                        both(i)
                else:
                    def ub(iv0, n):
                        for j in range(n):
                            both(iv0 + j)
                    tc.For_i_unrolled_general(
                        0, nsteps, 1, ub, max_unroll=unroll,
                        hint_engines=(mybir.EngineType.PE,))
                return outs

            # ---- L0 ----
            XT0 = input_side(0, xT, 4)
            h0f, h0b = recur(0, XT0)

            # ---- x1T assembly + L1 ----
            x1T = seq.tile([PB, 8, T], BF16, tag="x1T")
            nc.vector.tensor_copy(out=x1T[:, 0:4, :], in_=h0f[:, :, 1:T + 1])
            nc.vector.tensor_copy(out=x1T[:, 4:8, :], in_=h0b[:, :, 0:T])
            XT1 = input_side(1, x1T, 8)
            h1f, h1b = recur(1, XT1)

            # ---- hvT [100, 8, 256] bf16 and hv_n 2x[128, 800] bf16 ----
            hvT = seq.tile([PB, 8, T], BF16, tag="hvT")
            nc.vector.tensor_copy(out=hvT[:, 0:4, :], in_=h1f[:, :, 1:T + 1])
            nc.vector.tensor_copy(out=hvT[:, 4:8, :], in_=h1b[:, :, 0:T])
            hv_n = [seq.tile([128, BI], BF16, tag=f"hvn{i}", name=f"hvn{i}") for i in range(2)]
            for tcc in range(2):
                for c in range(8):
                    pt = psA.tile([128, PB], BF16, tag="ps", name="ptr2")
                    nc.tensor.transpose(
                        out=pt[:], in_=hvT[:, c, tcc * 128:(tcc + 1) * 128],
                        identity=idb_sb[0:PB, 0:PB])
                    nc.scalar.activation(
                        out=hv_n[tcc][:, c * PB:(c + 1) * PB], in_=pt[:], func=AF.Copy)

            # ---- AdbT [128, 13, 256] f32 (Ad + ab1) ----
            aW1d_sb = bigw.tile([PB, 8, KC13, 128], BF16, tag="bigw")
            dma(out=aW1d_sb[:], in_=d_aW1d[:])
            AdbT = seq.tile([128, KC13, T], F32, tag="AdbT")
            for mc in range(KC13):
                pa = psA.tile([128, T], F32, tag="ps", name="pa")
                for kc in range(8):
                    nc.tensor.matmul(out=pa[:], lhsT=aW1d_sb[:, kc, mc, :],
                                     rhs=hvT[:, kc, :], start=(kc == 0), stop=(kc == 7))
                nc.scalar.activation(out=AdbT[:, mc, :], in_=pa[:], func=AF.Identity,
                                     bias=ab1T_sb[:, mc:mc + 1], scale=1.0)

            # ---- hv_my [32, 800] -> hv_myT [100, 8, 32] -> AhT_my [128, 13, 32] ----
            hv_my = work.tile([RB, BI], BF16, tag="hvmy", bufs=1)
            for nh in range(2):
                ps = psB.tile([RB, 400], F32, tag="psb", name="psel")
                for tcc in range(2):
                    nc.tensor.matmul(out=ps[:], lhsT=isel_sb[:, tcc, :],
                                     rhs=hv_n[tcc][:, nh * 400:(nh + 1) * 400],
                                     start=(tcc == 0), stop=(tcc == 1))
                nc.scalar.activation(out=hv_my[:, nh * 400:(nh + 1) * 400],
                                     in_=ps[:], func=AF.Copy)
            hv_myT = work.tile([PB, 8, RB], BF16, tag="hvmyT", bufs=1)
            for c in range(8):
                pt = psA.tile([PB, RB], BF16, tag="ps", name="ptr3")
                nc.tensor.transpose(out=pt[:], in_=hv_my[:, c * PB:(c + 1) * PB],
                                    identity=idb_sb[0:RB, 0:RB])
                nc.scalar.activation(out=hv_myT[:, c, :], in_=pt[:], func=AF.Copy)
            aW1h_sb = bigw.tile([PB, 8, KC13, 128], BF16, tag="bigw")
            dma(out=aW1h_sb[:], in_=d_aW1h[:])
            AhT_my = work.tile([128, KC13, RB], F32, tag="AhTmy", bufs=1)
            for mc in range(KC13):
                pa = psB.tile([128, RB], F32, tag="psb", name="pah")
                for kc in range(8):
                    nc.tensor.matmul(out=pa[:], lhsT=aW1h_sb[:, kc, mc, :],
                                     rhs=hv_myT[:, kc, :], start=(kc == 0), stop=(kc == 7))
                nc.scalar.activation(out=AhT_my[:, mc, :], in_=pa[:], func=AF.Copy)

            # ---- arc grid: 32 rows x 256 cols ----
            for i in range(narc):
                pr = psB.tile([1, T], F32, tag="psb", name="prow")
                for mc in range(KC13):
                    th = work.tile([128, T], BF16, tag="th")
                    nc.scalar.activation(out=th[:], in_=AdbT[:, mc, :], func=AF.Tanh,
                                         bias=AhT_my[:, mc, i:i + 1], scale=1.0)
                    kk = 128 if mc < KC13 - 1 else 64
                    nc.tensor.matmul(out=pr[:], lhsT=aW2T_sb[0:kk, mc:mc + 1],
                                     rhs=th[0:kk, :], start=(mc == 0),
                                     stop=(mc == KC13 - 1))
                srow = work.tile([1, T], F32, tag="srow", name="srow", bufs=2)
                nc.scalar.activation(out=srow[:], in_=pr[:],
                                     func=AF.Identity, bias=ab2b_sb[:, 0:1], scale=1.0)
                mrowA = work.tile([1, T], F32, tag="mrowA", name="mrowA", bufs=2)
                mrowB = work.tile([1, T], F32, tag="mrowB", name="mrowB", bufs=2)
                dma(out=mrowA[:], in_=d_maskA[i:i + 1, :])
                dma(out=mrowB[:], in_=d_maskB[i:i + 1, :])
                sr2 = work.tile([1, T], F32, tag="sr2", name="sr2", bufs=2)
                nc.vector.tensor_tensor(out=sr2[:], in0=srow[:], in1=mrowA[:], op=OP.mult)
                sr3 = work.tile([1, T], F32, tag="sr3", name="sr3", bufs=2)
                nc.vector.tensor_tensor(out=sr3[:], in0=sr2[:], in1=mrowB[:], op=OP.add)
                dma(out=o_scores[i:i + 1, :], in_=sr3[:])

            # ---- labels (replicated) ----
            hv_arcs = [seq.tile([128, BI], BF16, tag=f"hva{i}", name=f"hva{i}") for i in range(2)]
            for tcc in range(2):
                for nh in range(2):
                    ps = psB.tile([128, 400], F32, tag="psb", name="psel2")
                    for tcp in range(2):
                        nc.tensor.matmul(out=ps[:], lhsT=arcsel_sb[:, tcp, tcc, :],
                                         rhs=hv_n[tcp][:, nh * 400:(nh + 1) * 400],
                                         start=(tcp == 0), stop=(tcp == 1))
                    nc.scalar.activation(out=hv_arcs[tcc][:, nh * 400:(nh + 1) * 400],
                                         in_=ps[:], func=AF.Copy)
            hv_arcsT = seq.tile([PB, 8, T], BF16, tag="hvaT")
            for tcc in range(2):
                for c in range(8):
                    pt = psA.tile([PB, 128], BF16, tag="ps", name="ptr4")
                    nc.tensor.transpose(out=pt[:], in_=hv_arcs[tcc][:, c * PB:(c + 1) * PB],
                                        identity=idb_sb[:])
                    nc.scalar.activation(
                        out=hv_arcsT[:, c, tcc * 128:(tcc + 1) * 128], in_=pt[:], func=AF.Copy)
            lW1h_sb = bigw.tile([PB, 8, KC13, 128], BF16, tag="bigw")
            dma(out=lW1h_sb[:], in_=d_lW1h[:])
            lW1d_sb = bigw.tile([PB, 8, KC13, 128], BF16, tag="bigw")
            dma(out=lW1d_sb[:], in_=d_lW1d[:])
            lhT = seq.tile([128, KC13, T], BF16, tag="lhT")
            for mc in range(KC13):
                pl = psA.tile([128, T], F32, tag="ps", name="pl")
                for kc in range(8):
                    nc.tensor.matmul(out=pl[:], lhsT=lW1h_sb[:, kc, mc, :],
                                     rhs=hvT[:, kc, :], start=(kc == 0), stop=False)
                for kc in range(8):
                    nc.tensor.matmul(out=pl[:], lhsT=lW1d_sb[:, kc, mc, :],
                                     rhs=hv_arcsT[:, kc, :], start=False, stop=(kc == 7))
                nc.scalar.activation(out=lhT[:, mc, :], in_=pl[:], func=AF.Tanh,
                                     bias=lb1T_sb[:, mc:mc + 1], scale=1.0)
            for tcc in range(2):
                plab = psB.tile([128, L], F32, tag="psb", name="plab")
                for mc in range(KC13):
                    nc.tensor.matmul(out=plab[:],
                                     lhsT=lhT[:, mc, tcc * 128:(tcc + 1) * 128],
                                     rhs=lW2T_sb[:, mc, :], start=(mc == 0),
                                     stop=(mc == KC13 - 1))
                lab = work.tile([128, L], F32, tag="lab", bufs=2)
                nc.vector.tensor_tensor(out=lab[:], in0=plab[:], in1=lb2b_sb[:], op=OP.add)
                dma(out=o_labels[tcc * 128:(tcc + 1) * 128, :], in_=lab[:])

    nc.finalize()
    return nc


def kernel(**inputs):
    sh, per_core, ab2 = _prep(inputs)
    nc = _build(ab2, unroll=8)
    in_maps = [{**sh, **per_core[c]} for c in range(NC_)]
    res = run_bass_kernel_spmd(nc, in_maps, core_ids=list(range(NC_)))
    scores = np.concatenate([res.results[c]["scores_blk"] for c in range(NC_)], axis=0)
    labels = res.results[0]["labels_o"]
    return scores.astype(np.float32), labels.astype(np.float32)


if __name__ == "__main__":
    import reference
    inputs = {k: np.asarray(v) for k, v in reference.setup_inputs().items()}
    s, lab = kernel(**inputs)
    print("scores", s.shape, "labels", lab.shape)
